# revision 4
# baseline (speedup 1.0000x reference)
"""Trainium2 Bass kernel for nn_MCM_37031208026850.

Strategy (8 NeuronCores, SPMD):
  - Shard the four 4096x512x4096 score GEMMs by query row: core r owns global
    rows [512r, 512(r+1)) (= batch b=r//2, image half r%2).
  - Each core projects its q-slices (cq, tq) and k-slices (ck, tk) locally,
    AllGathers the k projections (512x512 fp32 per core -> 512x4096 full),
    then streams k in 512-column chunks through the PE with q stationary.
  - The mean-over-HW term never touches the score matrix: mean = q @ ksum
    with ksum precomputed on host. Only the max needs the full scores; the
    vector engine max-reduces each PSUM tile as it is produced.
  - Tiny AllGather of the pooled co vectors (4x512 per core); every core
    computes all 16 softmax gates and selects/broadcasts the two gate images
    it needs with host-supplied one-hot matmuls.
  - The 1x1 value convs are folded on host (W512_64 @ Wv_c), computed
    full-batch per core; gating and the three 3x3 fusion convs run
    full-image per core (pair-duplicated), host takes core 2b's output.
  - Score/projection/conv matmuls run in float32r (1 cycle/row on TRN2,
    ~1.4e-4 rel err vs 2.5e-3 for bf16).
"""
import sys
sys.path.insert(0, "/opt/trn_rl_repo")

import numpy as np

import concourse.bass as bass
import concourse.mybir as mybir
import concourse.tile as tile
from concourse import bacc
from concourse import bass_utils
from concourse.masks import make_identity

B, C, H, W = 4, 512, 32, 32
HW = H * W
SCALE = 1.0 / C ** 0.5
NCORES = 8
P = 128
KT = C // P          # 4 k-tiles over channels
S = 512              # q-rows per core
NCH = 8              # global column chunks of 512
F32 = mybir.dt.float32
F32R = mybir.dt.float32r
AX = mybir.AxisListType.X
AF = mybir.ActivationFunctionType
MUL = mybir.AluOpType.mult
ADD = mybir.AluOpType.add


# ----------------------------------------------------------------------------
# host-side preparation
# ----------------------------------------------------------------------------

def host_prep(inputs):
    """Build the 8 per-core input maps from the full problem inputs."""
    xc = np.ascontiguousarray(inputs["xc"], dtype=np.float32)
    xt = np.ascontiguousarray(inputs["xt"], dtype=np.float32)
    f = lambda k: np.ascontiguousarray(inputs[k], dtype=np.float32)
    Wq_c, bq_c = f("Wq_c"), f("bq_c")
    Wk_c, bk_c = f("Wk_c"), f("bk_c")
    Wv_c, bv_c = f("Wv_c"), f("bv_c")
    Wq_t, bq_t = f("Wq_t"), f("bq_t")
    Wk_t, bk_t = f("Wk_t"), f("bk_t")
    W64, b64 = f("W512_64"), f("b512_64")
    W1, b1 = f("W1"), f("b1")
    W2, b2 = f("W2"), f("b2")
    W3, b3 = f("W3"), f("b3")

    xcG = np.ascontiguousarray(
        xc.reshape(B, C, HW).transpose(1, 0, 2).reshape(C, B * HW))
    xtT = np.ascontiguousarray(
        xt.transpose(2, 0, 1).reshape(C, B * HW))

    # ksum[c, kk*4+b]: column-sums of the k matrices per batch, /HW (mean),
    # computed from input sums so the score matrix is never needed.
    xc_sum = xc.sum(axis=(2, 3))                      # (B, C)
    xt_sum = xt.sum(axis=1)                           # (B, C)
    ks_ck = (Wk_c @ xc_sum.T + HW * bk_c[:, None]) / HW    # (C, B)
    ks_tk = (Wk_t @ xt_sum.T + HW * bk_t[:, None]) / HW    # (C, B)
    ksums = np.concatenate([ks_ck, ks_tk], axis=1)    # (C, 8) [kk*4+b]

    Wcv = W64 @ Wv_c                                  # (64, C)
    bcv = W64 @ bv_c                                  # (64,)
    wcv64 = np.ascontiguousarray(np.concatenate([Wcv, Wcv], axis=0).T)   # (C,128)
    wtv64 = np.ascontiguousarray(np.concatenate([W64, W64], axis=0).T)   # (C,128)
    bcv64 = np.concatenate([bcv, bcv]).reshape(P, 1)
    b64dup = np.concatenate([b64, b64]).reshape(P, 1)

    shared = {
        "wqct": np.ascontiguousarray(Wq_c.T),
        "wkct": np.ascontiguousarray(Wk_c.T),
        "wqtt": np.ascontiguousarray(Wq_t.T),
        "wktt": np.ascontiguousarray(Wk_t.T),
        "wcv64": wcv64,
        "wtv64": wtv64,
        "ksums": np.ascontiguousarray(ksums, dtype=np.float32),
        "bqc": np.ascontiguousarray(bq_c.reshape(KT, P).T),
        "bkc": np.ascontiguousarray(bk_c.reshape(KT, P).T),
        "bqt": np.ascontiguousarray(bq_t.reshape(KT, P).T),
        "bkt": np.ascontiguousarray(bk_t.reshape(KT, P).T),
        "bcv64": np.ascontiguousarray(bcv64, dtype=np.float32),
        "b64d": np.ascontiguousarray(b64dup, dtype=np.float32),
        "w1t": np.ascontiguousarray(W1.transpose(1, 2, 3, 0).reshape(P, 9, 64)),
        "w2t": np.ascontiguousarray(W2.transpose(1, 2, 3, 0).reshape(P, 9, 64)),
        "w3ta": np.ascontiguousarray(
            W3.transpose(1, 2, 3, 0).reshape(P, 9, 64)[:64]),
        "w3tb": np.ascontiguousarray(
            W3.transpose(1, 2, 3, 0).reshape(P, 9, 64)[64:]),
        "cb1": np.ascontiguousarray(b1.reshape(64, 1)),
        "cb2": np.ascontiguousarray(b2.reshape(64, 1)),
        "cb3": np.ascontiguousarray(b3.reshape(64, 1)),
    }

    in_maps = []
    for r in range(NCORES):
        myb = r // 2
        cols = slice(S * r, S * (r + 1))
        bcols = slice(HW * myb, HW * (myb + 1))
        sel1 = np.zeros((16, P), np.float32)   # T1 = [c_co; ct_co] combos 0,1
        sel2 = np.zeros((16, P), np.float32)   # T2 = [t_co; tc_co] combos 3,2
        for p in range(P):
            sel1[(0 if p < 64 else 1) * 4 + myb, p] = 1.0
            sel2[(3 if p < 64 else 2) * 4 + myb, p] = 1.0
        m = dict(shared)
        m["xcq"] = np.ascontiguousarray(xcG[:, cols])
        m["xtq"] = np.ascontiguousarray(xtT[:, cols])
        m["xcb"] = np.ascontiguousarray(xcG[:, bcols])
        m["xtb"] = np.ascontiguousarray(xtT[:, bcols])
        m["sel1"] = sel1
        m["sel2"] = sel2
        in_maps.append(m)
    return in_maps


# ----------------------------------------------------------------------------
# device program
# ----------------------------------------------------------------------------

def build_program(time_reps: int = 1, debug: bool = False):
    """Build + bacc-compile the SPMD Bass program.

    time_reps > 1 wraps the three compute segments in For_i loops (collectives
    stay outside) so wall-clock deltas between different reps counts measure
    pure per-iteration compute time.
    """
    nc = bacc.Bacc("TRN2", target_bir_lowering=False, debug=False,
                   num_devices=NCORES)

    def din(name, shape, dtype=F32R):
        return nc.dram_tensor(name, list(shape), dtype, kind="ExternalInput")

    xcq_d = din("xcq", (C, S)); xtq_d = din("xtq", (C, S))
    xcb_d = din("xcb", (C, HW)); xtb_d = din("xtb", (C, HW))
    wqct_d = din("wqct", (C, C)); wkct_d = din("wkct", (C, C))
    wqtt_d = din("wqtt", (C, C)); wktt_d = din("wktt", (C, C))
    wcv64_d = din("wcv64", (C, P)); wtv64_d = din("wtv64", (C, P))
    ksums_d = din("ksums", (C, 8))
    sel1_d = din("sel1", (16, P)); sel2_d = din("sel2", (16, P))
    bqc_d = din("bqc", (P, KT), F32); bkc_d = din("bkc", (P, KT), F32)
    bqt_d = din("bqt", (P, KT), F32); bkt_d = din("bkt", (P, KT), F32)
    bcv64_d = din("bcv64", (P, 1), F32); b64d_d = din("b64d", (P, 1), F32)
    w1t_d = din("w1t", (P, 9, 64)); w2t_d = din("w2t", (P, 9, 64))
    w3ta_d = din("w3ta", (64, 9, 64)); w3tb_d = din("w3tb", (64, 9, 64))
    cb1_d = din("cb1", (64, 1), F32); cb2_d = din("cb2", (64, 1), F32)
    cb3_d = din("cb3", (64, 1), F32)

    outp_d = nc.dram_tensor("outp", [64, HW], F32, kind="ExternalOutput")
    if debug:
        dbg_co_d = nc.dram_tensor("dbg_co", [4, S], F32, kind="ExternalOutput")
        dbg_gates_d = nc.dram_tensor("dbg_gates", [16, HW], F32,
                                     kind="ExternalOutput")
        dbg_cv_d = nc.dram_tensor("dbg_cv", [P, HW], F32, kind="ExternalOutput")
        dbg_cq_d = nc.dram_tensor("dbg_cq", [P, KT, S], F32,
                                  kind="ExternalOutput")

    KMAJ = "(kt p) n -> p kt n"

    with tile.TileContext(nc) as tc:
      with tc.tile_pool(name="consts", bufs=1) as cons, \
           tc.tile_pool(name="dram", bufs=1, space="DRAM") as dram:
        # ---------------- static loads ----------------
        xcq_sb = cons.tile([P, KT, S], F32R)
        xtq_sb = cons.tile([P, KT, S], F32R)
        xcb_sb = cons.tile([P, KT, HW], F32R)
        xtb_sb = cons.tile([P, KT, HW], F32R)
        nc.sync.dma_start(xcq_sb, xcq_d.ap().rearrange(KMAJ, p=P))
        nc.sync.dma_start(xtq_sb, xtq_d.ap().rearrange(KMAJ, p=P))
        nc.sync.dma_start(xcb_sb, xcb_d.ap().rearrange(KMAJ, p=P))
        nc.sync.dma_start(xtb_sb, xtb_d.ap().rearrange(KMAJ, p=P))
        wq_sb, wk_sb, bq_sb, bk_sb = [], [], [], []
        for w_d, b_d, wl, bl in ((wqct_d, bqc_d, wq_sb, bq_sb),
                                 (wqtt_d, bqt_d, wq_sb, bq_sb),
                                 (wkct_d, bkc_d, wk_sb, bk_sb),
                                 (wktt_d, bkt_d, wk_sb, bk_sb)):
            w_sb = cons.tile([P, KT, C], F32R, name=f"w_{w_d.name}")
            nc.sync.dma_start(w_sb, w_d.ap().rearrange(KMAJ, p=P))
            b_sb = cons.tile([P, KT], F32, name=f"b_{b_d.name}")
            nc.sync.dma_start(b_sb, b_d.ap())
            wl.append(w_sb)
            bl.append(b_sb)
        wcv_sb = cons.tile([P, KT, P], F32R)
        wtv_sb = cons.tile([P, KT, P], F32R)
        nc.sync.dma_start(wcv_sb, wcv64_d.ap().rearrange(KMAJ, p=P))
        nc.sync.dma_start(wtv_sb, wtv64_d.ap().rearrange(KMAJ, p=P))
        ksums_sb = cons.tile([P, KT, 8], F32R)
        nc.sync.dma_start(ksums_sb, ksums_d.ap().rearrange(KMAJ, p=P))
        sel1_sb = cons.tile([16, P], F32R)
        sel2_sb = cons.tile([16, P], F32R)
        nc.sync.dma_start(sel1_sb, sel1_d.ap())
        nc.sync.dma_start(sel2_sb, sel2_d.ap())
        bcv_sb = cons.tile([P, 1], F32); nc.sync.dma_start(bcv_sb, bcv64_d.ap())
        b64_sb = cons.tile([P, 1], F32); nc.sync.dma_start(b64_sb, b64d_d.ap())
        conv_w, conv_b = [], []
        for w_d, b_d in ((w1t_d, cb1_d), (w2t_d, cb2_d)):
            w_sb = cons.tile([P, 9, 64], F32R, name=f"cw_{w_d.name}")
            nc.sync.dma_start(w_sb, w_d.ap())
            b_sb = cons.tile([64, 1], F32, name=f"cb_{b_d.name}")
            nc.sync.dma_start(b_sb, b_d.ap())
            conv_w.append(w_sb)
            conv_b.append(b_sb)
        w3a_sb = cons.tile([64, 9, 64], F32R)
        w3b_sb = cons.tile([64, 9, 64], F32R)
        nc.sync.dma_start(w3a_sb, w3ta_d.ap())
        nc.sync.dma_start(w3b_sb, w3tb_d.ap())
        cb3_sb = cons.tile([64, 1], F32)
        nc.sync.dma_start(cb3_sb, cb3_d.ap())
        ident = cons.tile([P, P], F32)
        make_identity(nc, ident)

        # persistent intermediates
        q_sb = [cons.tile([P, KT, S], F32R, name=f"q{i}") for i in range(2)]
        kslice_sb = [cons.tile([P, KT, S], F32R, name=f"ksl{i}")
                     for i in range(2)]
        cv_sb = cons.tile([P, HW], F32)
        tv_sb = cons.tile([P, HW], F32)
        strip = cons.tile([P, KT, 4, 4, 2], F32)   # [i, mi, combo, b, h]
        co_sb = cons.tile([P, KT, 4], F32)         # [i, mi, combo]
        co_row = cons.tile([4, S], F32)            # [combo, i]
        gates_sb = cons.tile([16, HW], F32)
        rmax = cons.tile([16, 1], F32)
        negmax = cons.tile([16, 1], F32)
        expacc = cons.tile([16, 1], F32)
        rsum = cons.tile([16, 1], F32)
        gates_n = cons.tile([16, HW], F32R)
        T1 = cons.tile([P, H + 2, W + 2], F32R)
        T2 = cons.tile([P, H + 2, W + 2], F32R)
        T3a = cons.tile([64, H + 2, W + 2], F32R)
        T3b = cons.tile([64, H + 2, W + 2], F32R)
        out_sb = cons.tile([64, H, W], F32)
        zerot = cons.tile([P, H + 2, W + 2], F32)
        nc.vector.memset(zerot, 0.0)
        nc.vector.tensor_copy(T1, zerot)
        nc.vector.tensor_copy(T2, zerot)
        nc.vector.tensor_copy(T3a, zerot[:64])
        nc.vector.tensor_copy(T3b, zerot[:64])

        kslice_dram = [dram.tile([C, S], F32R, name=f"ksd{i}")
                       for i in range(2)]
        ag_out = [dram.tile([NCORES * C, S], F32R, addr_space="Shared",
                            name=f"ag{i}") for i in range(2)]
        co_dram = dram.tile([4, S], F32)
        co_all = dram.tile([NCORES * 4, S], F32, addr_space="Shared")

        rep = (lambda: tc.For_i(0, time_reps, 1)) if time_reps > 1 else None

        # ---------------- segment 1: projections + values ----------------
        import contextlib

        with tc.tile_pool(name="pj", bufs=4, space="PSUM") as pj:
          with rep() if rep else contextlib.nullcontext():
            # k projections first so the AllGathers launch early
            for kk in range(2):
                rhs = (xcq_sb, xtq_sb)[kk]
                for m in range(KT):
                    pq = pj.tile([P, S], F32, tag="pq", name="pq")
                    for kt in range(KT):
                        nc.tensor.matmul(pq, wk_sb[kk][:, kt, P * m:P * (m + 1)],
                                         rhs[:, kt], start=(kt == 0),
                                         stop=(kt == KT - 1))
                    nc.vector.tensor_scalar_add(kslice_sb[kk][:, m, :], pq,
                                                bk_sb[kk][:, m:m + 1])
                nc.sync.dma_start(
                    kslice_dram[kk].opt().rearrange(KMAJ, p=P), kslice_sb[kk])
            # q projections
            for qi in range(2):
                rhs = (xcq_sb, xtq_sb)[qi]
                for m in range(KT):
                    pq = pj.tile([P, S], F32, tag="pq", name="pq")
                    for kt in range(KT):
                        nc.tensor.matmul(pq, wq_sb[qi][:, kt, P * m:P * (m + 1)],
                                         rhs[:, kt], start=(kt == 0),
                                         stop=(kt == KT - 1))
                    nc.vector.tensor_scalar_add(q_sb[qi][:, m, :], pq,
                                                bq_sb[qi][:, m:m + 1])
            # folded 64-channel value projections (duplicated to 128 partitions)
            for vi, (wv, vt) in enumerate(((wcv_sb, cv_sb), (wtv_sb, tv_sb))):
                for nh in range(2):
                    pv = pj.tile([P, 512], F32, tag="pq", name="pv")
                    for kt in range(KT):
                        nc.tensor.matmul(
                            pv, wv[:, kt],
                            (xcb_sb, xtb_sb)[vi][:, kt, 512 * nh:512 * (nh + 1)],
                            start=(kt == 0), stop=(kt == KT - 1))
                    if vi == 0:
                        nc.vector.tensor_scalar_add(
                            vt[:, 512 * nh:512 * (nh + 1)], pv, bcv_sb)
                    else:
                        nc.vector.tensor_copy(
                            vt[:, 512 * nh:512 * (nh + 1)], pv)

        # ---------------- k AllGathers ----------------
        for kk in range(2):
            nc.gpsimd.collective_compute(
                "AllGather", mybir.AluOpType.bypass,
                replica_groups=[list(range(NCORES))],
                ins=[kslice_dram[kk].opt()], outs=[ag_out[kk].opt()])

        # ---------------- segment 2: scores + co ----------------
        with tc.tile_pool(name="sc", bufs=6, space="PSUM") as sc, \
             tc.tile_pool(name="fin", bufs=1, space="PSUM") as fin, \
             tc.tile_pool(name="kch", bufs=3) as kch:
          with rep() if rep else contextlib.nullcontext():
            for kk in range(2):
                for ch in range(NCH):
                    kchunk = kch.tile([P, KT, 512], F32R, tag="kch",
                                      name="kchunk")
                    nc.sync.dma_start(
                        kchunk,
                        ag_out[kk][C * ch:C * (ch + 1), :].rearrange(KMAJ, p=P))
                    b_, h_ = ch // 2, ch % 2
                    for mi in range(KT):
                        for qi in range(2):
                            ps = sc.tile([P, 512], F32, tag="ps", name="ps")
                            for kt in range(KT):
                                nc.tensor.matmul(
                                    ps, q_sb[qi][:, kt, P * mi:P * (mi + 1)],
                                    kchunk[:, kt], start=(kt == 0),
                                    stop=(kt == KT - 1))
                            nc.vector.reduce_max(
                                strip[:, mi, 2 * qi + kk, b_, h_:h_ + 1],
                                ps, axis=AX)
            # assemble co per m-tile
            for mi in range(KT):
                for qi in range(2):
                    pm = fin.tile([P, 8], F32, tag="pm", name="pm")
                    for kt in range(KT):
                        nc.tensor.matmul(pm,
                                         q_sb[qi][:, kt, P * mi:P * (mi + 1)],
                                         ksums_sb[:, kt], start=(kt == 0),
                                         stop=(kt == KT - 1))
                    mx = cons.tile([P, 2, 4], F32, name="mx", tag="mx")
                    nc.vector.reduce_max(mx, strip[:, mi, 2 * qi:2 * qi + 2],
                                         axis=AX)
                    cmb = cons.tile([P, 2, 4], F32, name="cmb", tag="cmb")
                    nc.vector.tensor_tensor(
                        cmb, mx, pm.rearrange("p (k b) -> p k b", k=2), ADD)
                    nc.vector.reduce_sum(co_sb[:, mi, 2 * qi:2 * qi + 2], cmb,
                                         axis=AX)
                ptr = fin.tile([P, P], F32, tag="ptr", name="ptr")
                nc.tensor.transpose(ptr[:4, :], co_sb[:, mi, :], ident)
                nc.vector.tensor_copy(co_row[:, P * mi:P * (mi + 1)],
                                      ptr[:4, :])
            nc.sync.dma_start(co_dram.opt(), co_row)

        # ---------------- co AllGather ----------------
        nc.gpsimd.collective_compute(
            "AllGather", mybir.AluOpType.bypass,
            replica_groups=[list(range(NCORES))],
            ins=[co_dram.opt()], outs=[co_all.opt()])

        # ---------------- segment 3: gates + fusion convs ----------------
        with tc.tile_pool(name="g", bufs=2, space="PSUM") as g:
          with rep() if rep else contextlib.nullcontext():
            co_view = co_all.opt().rearrange("(b h c) i -> c b h i", b=4,
                                             h=2, c=4)
            for cmb_i in range(4):
                nc.sync.dma_start(
                    gates_sb[4 * cmb_i:4 * (cmb_i + 1), :].rearrange(
                        "p (h i) -> p h i", h=2),
                    co_view[cmb_i])
            nc.vector.reduce_max(rmax, gates_sb, axis=AX)
            nc.vector.tensor_scalar_mul(negmax, rmax, -SCALE)
            expg = cons.tile([16, HW], F32, name="expg")
            nc.scalar.activation(expg, gates_sb, AF.Exp, bias=negmax,
                                 scale=SCALE, accum_out=expacc)
            nc.vector.reciprocal(rsum, expacc)
            nc.vector.tensor_scalar_mul(gates_n, expg, rsum)
            # gate selection + gating, into padded conv inputs
            for ti, (sel, val, T) in enumerate(
                    ((sel1_sb, cv_sb, T1), (sel2_sb, tv_sb, T2))):
                for nh in range(2):
                    pbg = g.tile([P, 512], F32, tag="pbg", name="pbg")
                    nc.tensor.matmul(pbg, sel,
                                     gates_n[:, 512 * nh:512 * (nh + 1)],
                                     start=True, stop=True)
                    reg = T[:, 1 + 16 * nh:17 + 16 * nh, 1:33]
                    nc.vector.tensor_tensor(
                        reg, pbg.rearrange("p (y x) -> p y x", y=16),
                        val[:, 512 * nh:512 * (nh + 1)].rearrange(
                            "p (y x) -> p y x", y=16), MUL)
                    nc.vector.tensor_scalar_add(reg, reg, b64_sb)
            # conv1/conv2: 128-ch input, 64-ch output into T3a/T3b interiors
            for srcT, wi, dstT in ((T1, 0, T3a), (T2, 1, T3b)):
                for cy in range(4):
                    pc = g.tile([64, 8, 32], F32, tag="pc", name="pc")
                    for tap in range(9):
                        dy, dx = tap // 3, tap % 3
                        nc.tensor.matmul(
                            pc, conv_w[wi][:, tap, :],
                            srcT[:, 8 * cy + dy:8 * cy + dy + 8, dx:dx + 32],
                            start=(tap == 0), stop=(tap == 8))
                    nc.scalar.activation(
                        dstT[:, 1 + 8 * cy:9 + 8 * cy, 1:33], pc, AF.Relu,
                        bias=conv_b[wi], scale=1.0)
            # conv3: contraction split into two 64-channel halves
            for cy in range(4):
                pc = g.tile([64, 8, 32], F32, tag="pc", name="pc")
                for hi, (wh, Th) in enumerate(((w3a_sb, T3a), (w3b_sb, T3b))):
                    for tap in range(9):
                        dy, dx = tap // 3, tap % 3
                        nc.tensor.matmul(
                            pc, wh[:, tap, :],
                            Th[:, 8 * cy + dy:8 * cy + dy + 8, dx:dx + 32],
                            start=(hi == 0 and tap == 0),
                            stop=(hi == 1 and tap == 8))
                nc.scalar.activation(out_sb[:, 8 * cy:8 * (cy + 1), :], pc,
                                     AF.Relu, bias=cb3_sb, scale=1.0)
            nc.sync.dma_start(outp_d.ap().rearrange("o (y x) -> o y x", y=H),
                              out_sb)
            if debug:
                nc.sync.dma_start(dbg_co_d.ap(), co_row)
                nc.sync.dma_start(dbg_gates_d.ap(), gates_n.bitcast(F32))
                nc.sync.dma_start(dbg_cv_d.ap(), cv_sb)
                nc.sync.dma_start(
                    dbg_cq_d.ap(),
                    q_sb[0].bitcast(F32))

    nc.compile()
    return nc


# ----------------------------------------------------------------------------
# entry point
# ----------------------------------------------------------------------------

_CACHE = {}


def _get_nc():
    if "nc" not in _CACHE:
        _CACHE["nc"] = build_program()
    return _CACHE["nc"]


def kernel(**inputs) -> np.ndarray:
    nc = _get_nc()
    in_maps = host_prep(inputs)
    res = bass_utils.run_bass_kernel_spmd(nc, in_maps,
                                          core_ids=list(range(NCORES)))
    out = np.empty((B, 64, H, W), np.float32)
    for b in range(B):
        out[b] = res.results[2 * b]["outp"].reshape(64, H, W)
    return out


if __name__ == "__main__":
    # smoke test with random inputs
    rng = np.random.default_rng(0)
    d = {
        "xc": rng.standard_normal((B, C, H, W), np.float32),
        "xt": rng.standard_normal((B, HW, C), np.float32),
    }
    for nm, o in (("q_c", C), ("k_c", C), ("v_c", C), ("q_t", C), ("k_t", C)):
        d[f"W{nm}"] = rng.standard_normal((o, C), np.float32) * 0.02
        d[f"b{nm}"] = np.zeros(o, np.float32)
    d["W512_64"] = rng.standard_normal((64, C), np.float32) * 0.02
    d["b512_64"] = np.zeros(64, np.float32)
    for i in (1, 2, 3):
        d[f"W{i}"] = rng.standard_normal((64, 128, 3, 3), np.float32) * 0.02
        d[f"b{i}"] = np.zeros(64, np.float32)
    out = kernel(**d)
    print("out", out.shape, out.dtype, np.abs(out).max())
